# revision 5
# baseline (speedup 1.0000x reference)
"""Int8 Llama MLP (W8A8) on 8 Trainium2 NeuronCores — v5.

Same TP-over-I strategy as v1, restructured for HW efficiency:
  - gate/up weights ship as int8 ko-slabs streamed through a small FIFO
    ring and dequantized to bf16 on the mostly-idle DVE (halves the
    dominant per-block DMA stream; DMA concurrency steals PE time).
  - down weights (down_wT shard) ship as bf16 and stay RESIDENT in SBUF:
    loaded once, no per-block re-DMA/dequant; each down stationary (ht
    token-slice) feeds 8 consecutive matmuls (full H=4096 via all 8 PSUM
    banks) -> 4x fewer LDWEIGHTS in the down phase.
  - one unified PSUM pool (8 banks): gate/up use 4 banks double-buffered
    across i-tile parity (postproc of tile i overlaps chains of i+1);
    the down phase uses all 8.
  - output staged in [128, 2048] bf16 halves (16 output DMAs per block
    instead of 64).

Each core returns its bf16 partial [NB, TB, H]; the host sums in f32.
"""

import numpy as np
import ml_dtypes

import concourse.bass as bass
import concourse.mybir as mybir
import concourse.tile as tile
from concourse import bacc
from concourse.bass_utils import run_bass_kernel_spmd

T, H, I = 8192, 4096, 11008
NCORES = 8
IP = 11264                 # I zero-padded to a multiple of 8*128
ISH = IP // NCORES         # 1408 intermediate rows per core
NI = ISH // 128            # 11 partition tiles of the I-shard
KO = H // 128              # 32 k-chunks for gate/up contraction
KS = 16                    # ko-slab size for gate/up weight DMA
NSLAB = KO // KS           # 2 slabs per (i-tile, gate/up)
TB = 1024                  # token block
NB = T // TB               # 8 token blocks
NT = TB // 128             # 8 token sub-tiles per block (down stationary)
HG = 512                   # H chunk per down matmul (one PSUM bank)
NHG = H // HG              # 8 H chunks = 8 PSUM banks per token sub-tile

F32 = mybir.dt.float32
F16 = mybir.dt.float16
BF16 = mybir.dt.bfloat16
MAGIC = 12582912.0         # 1.5 * 2^23: float32 round-to-nearest-even trick

_prog_cache = {}


def _build_program(share_x: bool, gsc: float, usc_over_dis: float, dsc: float):
    key = (share_x, gsc, usc_over_dis, dsc)
    if key in _prog_cache:
        return _prog_cache[key]

    nc = bacc.Bacc(None)
    xq = nc.declare_dram_parameter("xq", [NB, 128, KO, TB], BF16, isOutput=False)
    if share_x:
        xq2 = xq
    else:
        xq2 = nc.declare_dram_parameter("xq2", [NB, 128, KO, TB], BF16, isOutput=False)
    # gate/up weights, bf16 ko-slabs: [NI, 2(g/u), NSLAB, 128, KS, 128]
    wgu = nc.declare_dram_parameter("wgu", [NI, 2, NSLAB, 128, KS, 128],
                                    mybir.dt.int8, isOutput=False)
    # down_wT shard, bf16: [NI, 128, H]
    wdt = nc.declare_dram_parameter("wdt", [NI, 128, H], BF16, isOutput=False)
    rs_in = nc.declare_dram_parameter("out", [NB, TB, H], BF16, isOutput=True)

    ACT = mybir.ActivationFunctionType
    ALU = mybir.AluOpType

    with tile.TileContext(nc) as tc:
        with (
            tc.tile_pool(name="px", bufs=1) as px,
            tc.tile_pool(name="pw8", bufs=3) as pw8,
            tc.tile_pool(name="pw", bufs=3) as pw,
            tc.tile_pool(name="pdw", bufs=1) as pdw,
            tc.tile_pool(name="pht", bufs=NI) as pht,
            tc.tile_pool(name="ptmp", bufs=1) as ptmp,
            tc.tile_pool(name="pout", bufs=1) as pout,
            tc.tile_pool(name="ps", bufs=1, space="PSUM") as ps,
        ):
            qs = [nc.scalar, nc.sync]

            # x DMA in 1MB chunks round-robined over the queues
            xstep = max(1, KO // 8)
            nxch = KO // xstep

            def load_x(b):
                x_sb = px.tile([128, KO, TB], BF16, tag="x", name="x_sb")
                for c in range(nxch):
                    qs[c % 2].dma_start(x_sb[:, c * xstep:(c + 1) * xstep, :],
                                        xq[b, :, c * xstep:(c + 1) * xstep, :])
                if share_x:
                    x2_sb = x_sb
                else:
                    x2_sb = px.tile([128, KO, TB], BF16, tag="x2", name="x2_sb")
                    for c in range(nxch):
                        qs[c % 2].dma_start(x2_sb[:, c * xstep:(c + 1) * xstep, :],
                                            xq2[b, :, c * xstep:(c + 1) * xstep, :])
                return x_sb, x2_sb

            # gate/up weight slab FIFO: global use-order, prefetched ahead.
            slab_seq = [(i, g, s)
                        for b in range(NB)
                        for i in range(NI)
                        for g in range(2)
                        for s in range(NSLAB)]
            slab_fifo = []
            slab_pos = [0]

            def prefetch_slab():
                if slab_pos[0] < len(slab_seq):
                    i, g, s = slab_seq[slab_pos[0]]
                    w8_sb = pw8.tile([128, KS, 128], mybir.dt.int8, tag="w8",
                                     name="w8_sb")
                    qs[slab_pos[0] % 2].dma_start(w8_sb[:], wgu[i, g, s])
                    w_sb = pw.tile([128, KS, 128], BF16, tag="w", name="w_sb")
                    nc.vector.tensor_copy(w_sb[:], w8_sb[:])
                    slab_fifo.append(w_sb)
                    slab_pos[0] += 1

            def next_slab():
                prefetch_slab()
                return slab_fifo.pop(0)

            # resident down weights: load once (bf16 straight from DRAM)
            wdt_sb = []
            for k in range(NI):
                wtile = pdw.tile([128, H], BF16, tag=f"wdt{k}", name=f"wdt{k}")
                qs[k % 2].dma_start(wtile[:], wdt[k])
                wdt_sb.append(wtile)

            x_cur = load_x(0)
            prefetch_slab()
            prefetch_slab()

            for b in range(NB):
                x_sb, x2_sb = x_cur
                xs = [x_sb, x2_sb]

                ht_tiles = []
                for i in range(NI):
                    par = 4 * (i % 2)       # PSUM bank parity for this i
                    gu_ps = []
                    for g in range(2):      # 0=gate, 1=up
                        ps_h = [ps.tile([128, 512], F32, tag=f"b{par + 2 * g + h}",
                                        name=f"ps{g}{h}")
                                for h in range(2)]
                        for s in range(NSLAB):
                            w_sb = next_slab()
                            for ko in range(KS):
                                wt = w_sb[:, ko, :]
                                kabs = s * KS + ko
                                for h in range(2):
                                    nc.tensor.matmul(
                                        ps_h[h][:], wt,
                                        xs[g][:, kabs, h * 512:(h + 1) * 512],
                                        start=(kabs == 0), stop=(kabs == KO - 1))
                        gu_ps.append(ps_h)

                    # hidden = silu(f16(g*gsc)) * (u*usc/dis), round+clip int8
                    g_ps, u_ps = gu_ps
                    ht_i = pht.tile([128, TB], BF16, tag="ht", name="ht_i")
                    h32 = ptmp.tile([128, TB], F16, tag="h32", name="h32")
                    for h in range(2):
                        sl = slice(h * 512, (h + 1) * 512)
                        sl16 = ptmp.tile([128, 512], F16, tag=f"sl16{h}", name="sl16")
                        nc.scalar.activation(sl16[:], g_ps[h][:], ACT.Silu, scale=gsc)
                        nc.vector.scalar_tensor_tensor(h32[:, sl], u_ps[h][:],
                                                       usc_over_dis,
                                                       sl16[:], ALU.mult, ALU.mult)
                    c32 = ptmp.tile([128, TB], F16, tag="c32", name="c32")
                    nc.vector.tensor_scalar(c32[:], h32[:], -128.49, 127.49,
                                            ALU.max, ALU.min)
                    nc.vector.tensor_scalar(ht_i[:], c32[:], MAGIC, MAGIC,
                                            ALU.add, ALU.subtract)
                    ht_tiles.append(ht_i)

                # down proj, transposed: stationary = ht[k][:, 128-token
                # slice] feeds 8 consecutive matmuls (full H via 8 banks).
                # next block's x chunks are issued spread across the t-loop
                # to avoid a DMA burst.
                x_next = None
                if b + 1 < NB:
                    if share_x:
                        x_next = px.tile([128, KO, TB], BF16, tag="x", name="x_sb")
                    else:
                        x_cur = load_x(b + 1)
                for t in range(NT):
                    if x_next is not None and t < nxch:
                        qs[t % 2].dma_start(
                            x_next[:, t * xstep:(t + 1) * xstep, :],
                            xq[b + 1, :, t * xstep:(t + 1) * xstep, :])
                    tsl = slice(t * 128, (t + 1) * 128)
                    d_ps = [ps.tile([128, HG], F32, tag=f"b{g}", name=f"d{g}")
                            for g in range(NHG)]
                    for k in range(NI):
                        st = ht_tiles[k][:, tsl]
                        for g in range(NHG):
                            csl = slice(g * HG, (g + 1) * HG)
                            nc.tensor.matmul(d_ps[g][:], st, wdt_sb[k][:, csl],
                                             start=(k == 0), stop=(k == NI - 1))
                    for half in range(2):
                        o_sb = pout.tile([128, 2048], BF16, tag="o", name="o_sb")
                        for g in range(4):
                            nc.scalar.activation(o_sb[:, g * HG:(g + 1) * HG],
                                                 d_ps[4 * half + g][:], ACT.Copy,
                                                 scale=dsc)
                        nc.sync.dma_start(
                            rs_in[b, t * 128:(t + 1) * 128,
                                  half * 2048:(half + 1) * 2048], o_sb[:])

                if b + 1 < NB:
                    x_cur = (x_next, x_next)

    nc.finalize()
    _prog_cache[key] = nc
    return nc


def _quant_tile_x(x: np.ndarray, scale: float) -> np.ndarray:
    """clip(round(x/scale)) -> tiled [NB, 128, KO, TB] bf16 (exact ints)."""
    q = np.clip(np.round(x / np.float32(scale)), -128, 127).astype(np.float32)
    return np.ascontiguousarray(
        q.reshape(NB, TB, KO, 128).transpose(0, 3, 2, 1)
    ).astype(ml_dtypes.bfloat16)


def _prepare_in_maps(x, gate_w, up_w, down_w, gis, uis, share_x):
    xq = _quant_tile_x(np.asarray(x, np.float32), gis)
    xq2 = None if share_x else _quant_tile_x(np.asarray(x, np.float32), uis)

    # zero-pad I (11008 -> 11264)
    gw = np.zeros((IP, H), np.int8); gw[:I] = np.asarray(gate_w)
    uw = np.zeros((IP, H), np.int8); uw[:I] = np.asarray(up_w)
    dwt = np.zeros((IP, H), np.int8); dwt[:I] = np.asarray(down_w).T

    in_maps = []
    for c in range(NCORES):
        i0, i1 = c * ISH, (c + 1) * ISH
        # [NI, 2, NSLAB, 128kc, KS, 128icol]
        wgu_c = np.empty((NI, 2, NSLAB, 128, KS, 128), np.int8)
        for g, w in ((0, gw), (1, uw)):
            # w[i0:i1]: [NI*128ic, NSLAB*KS*128kc]
            wc = w[i0:i1].reshape(NI, 128, NSLAB, KS, 128)
            # -> [NI, NSLAB, 128kc, KS, 128ic]
            wgu_c[:, g] = wc.transpose(0, 2, 4, 3, 1)
        wdt_c = np.ascontiguousarray(
            dwt[i0:i1].reshape(NI, 128, H)).astype(ml_dtypes.bfloat16)
        m = {"xq": xq, "wgu": wgu_c, "wdt": wdt_c}
        if not share_x:
            m["xq2"] = xq2
        in_maps.append(m)
    return in_maps


def kernel(x, gate_w, up_w, down_w,
           gate_in_scale, gate_w_scale,
           up_in_scale, up_w_scale,
           down_in_scale, down_w_scale):
    gis = float(gate_in_scale)
    uis = float(up_in_scale)
    dis = float(down_in_scale)
    gsc = float(np.float32(gis) * np.float32(gate_w_scale))
    usc = float(np.float32(uis) * np.float32(up_w_scale))
    dsc = float(np.float32(dis) * np.float32(down_w_scale))
    share_x = (np.float32(gis) == np.float32(uis))

    nc = _build_program(share_x, gsc, usc / dis, dsc)
    in_maps = _prepare_in_maps(x, gate_w, up_w, down_w, gis, uis, share_x)

    res = run_bass_kernel_spmd(nc, in_maps, list(range(NCORES)))

    acc = np.zeros((NB, TB, H), np.float32)
    for c in range(NCORES):
        acc += np.asarray(res.results[c]["out"]).astype(np.float32)
    return acc.reshape(T, H)
